# revision 3
# baseline (speedup 1.0000x reference)
"""GCN encoder layer (degree-normalized message passing + BN inference) on 8 Trainium2 cores.

Math (see reference):
    t = X @ W + b                                  [N, H]
    deg = out-degree by src                        [N]
    isd = deg ** -0.5
    nb_sum[i]  = isd[i] * sum_{e: src=i} isd[dst_e] * t[dst_e]
    src_mean   = deg * t            (segment_mean(deg[src]*t[src]) simplifies exactly)
    agg = 0.5*nb_sum + 0.5*src_mean
    out = (agg - mean) * rsqrt(var+eps) * gamma + beta

Strategy (edge-parallel, sharded by src bucket -> no cross-core reduction):
  - Src nodes are assigned to 392 (core, window) buckets of 128 slots each by
    snake order on out-degree, equalizing edges per bucket (and per core).
  - The whole pipeline runs in bf16 (PSUM accumulation fp32); tolerance 2e-2.
  - The dst gather is descriptor-rate limited (~0.38 rows/ns over 4 SWDGE
    queues), so descriptors are the currency: each descriptor fetches a PAIR
    of adjacent bf16 table rows (elem 512B).  A per-core table permutation
    places nodes that are co-used in the same window in the same pair-slot, so
    one descriptor serves up to two edges (G0 = even row, G1 = odd row).
    Pair-slot indices (< 25088) fit int16 with no table split.
  - Scatter-add via one-hot matmuls on the PE:  Z_T[f, s] += G0.T @ O0 +
    G1.T @ O1 per descriptor batch, chained in one PSUM bank per window.
    O_side = (iota == srcl_side) * scl_side built in ONE fused DVE
    tensor_scalar (op0=is_equal, op1=mult) per batch side.
  - Source term: host pre-scales own rows by 0.5*deg; an HWDGE transpose-DMA
    delivers them feature-major, added via a second W matmul.
  - nb_T = W.T @ (zt) + W.T @ xT accumulated in PSUM; BN affine on ACT while
    copying to the output slab (feature-major; transposed on the host).
"""

import math
import numpy as np
import ml_dtypes

N_CORES = 8
P = 128
F = 128
H = 128
BN_EPS = 1e-3
NW = 49                 # windows per core
NPC = NW * P            # 6272 src slots per core
NTOT = N_CORES * NPC    # 50176 node slots (incl. padding)
CHB = 16                # gather chunk size in batches (16*128 descs)

_CACHE = {}


def _wrap16(arr):
    """dma_gather index layout: unwrapped[i] = w[i%16, i//16], replicated x8."""
    w = arr.reshape(-1, 16).T.copy()
    return np.ascontiguousarray(np.tile(w, (8, 1)))


def _build_host_data(edge_pairs, node_features):
    n_nodes = node_features.shape[0]
    src = np.asarray(edge_pairs[:, 0], dtype=np.int64)
    dst = np.asarray(edge_pairs[:, 1], dtype=np.int64)
    deg = np.bincount(src, minlength=n_nodes).astype(np.float64)

    # ---- bucket assignment: snake on degree over 392 buckets of 128 ----
    nb_buckets = N_CORES * NW
    order = np.argsort(-deg, kind="stable")          # node ids, deg desc
    bucket_of_rank = np.empty(NTOT, dtype=np.int64)
    fwd = np.arange(nb_buckets)
    for r in range(P):
        row = fwd if r % 2 == 0 else fwd[::-1]
        bucket_of_rank[r * nb_buckets:(r + 1) * nb_buckets] = row
    node_bucket = np.full(NTOT, -1, dtype=np.int64)
    node_slot = np.full(NTOT, -1, dtype=np.int64)
    padded_nodes = np.concatenate([order, np.arange(n_nodes, NTOT)])
    node_bucket[padded_nodes] = bucket_of_rank
    # slot within bucket = rank of appearance
    cnt = np.zeros(nb_buckets, dtype=np.int64)
    slot_of_rank = np.empty(NTOT, dtype=np.int64)
    for i, b in enumerate(bucket_of_rank):
        slot_of_rank[i] = cnt[b]
        cnt[b] += 1
    node_slot[padded_nodes] = slot_of_rank
    node_core = node_bucket % N_CORES
    node_win = node_bucket // N_CORES

    # inverse map for output unshard: node -> global row in [NTOT]
    node_row = node_core * NPC + node_win * P + node_slot

    # ---- per-edge metadata ----
    core_e = node_core[src]
    win_e = node_win[src]
    srcl_e = node_slot[src]
    with np.errstate(divide="ignore"):
        scl_e = (0.5 / np.sqrt(deg[src] * deg[dst])).astype(np.float32)

    IDXs, S0s, C0s, S1s, C1s = [], [], [], [], []
    tabperm = np.empty((N_CORES, NTOT), dtype=np.int64)
    ndesc_cw = np.zeros((N_CORES, NW), dtype=np.int64)
    per_core = []
    for c in range(N_CORES):
        m = core_e == c
        wc, dc, sc, cc = win_e[m], dst[m], srcl_e[m], scl_e[m]
        # ---- greedy pair matching over windows ----
        partner = np.full(n_nodes, -1, dtype=np.int64)
        o = np.lexsort((dc, wc))
        wc, dc, sc, cc = wc[o], dc[o], sc[o], cc[o]
        wbounds = np.searchsorted(wc, np.arange(NW + 1))
        for w in range(NW):
            dw = np.unique(dc[wbounds[w]:wbounds[w + 1]])
            cand = dw[partner[dw] < 0]
            k = len(cand) // 2 * 2
            a, b = cand[0:k:2], cand[1:k:2]
            partner[a] = b
            partner[b] = a
        # ---- per-core table permutation: pairs adjacent ----
        nodes = np.arange(n_nodes)
        isp = partner >= 0
        a_nodes = nodes[isp & (nodes < partner)]
        singles = nodes[~isp]
        npad = NTOT - n_nodes
        ordert = np.empty(NTOT, dtype=np.int64)
        na = len(a_nodes)
        ordert[0:2 * na:2] = a_nodes
        ordert[1:2 * na:2] = partner[a_nodes]
        ordert[2 * na:2 * na + len(singles)] = singles
        ordert[2 * na + len(singles):] = np.arange(n_nodes, NTOT)
        tabperm[c] = ordert
        pos = np.empty(NTOT, dtype=np.int64)
        pos[ordert] = np.arange(NTOT)
        # ---- descriptor assembly per window ----
        kk = pos[dc] // 2
        side = pos[dc] % 2
        per_core.append((wc, kk, side, sc, cc, wbounds))

    # window batch counts (shared across cores -> max)
    for c in range(N_CORES):
        wc, kk, side, sc, cc, wbounds = per_core[c]
        for w in range(NW):
            lo, hi = wbounds[w], wbounds[w + 1]
            if hi == lo:
                continue
            k_w, s_w = kk[lo:hi], side[lo:hi]
            o2 = np.lexsort((s_w, k_w))
            k_s, s_s = k_w[o2], s_w[o2]
            uk, first = np.unique(k_s, return_index=True)
            c0 = np.add.reduceat((s_s == 0).astype(np.int64), first)
            c1 = np.add.reduceat((s_s == 1).astype(np.int64), first)
            ndesc_cw[c, w] = np.maximum(c0, c1).sum()
    nb = np.ceil(ndesc_cw.max(axis=0) / P).astype(np.int64)   # [NW]
    NB = int(nb.sum())
    cum = np.concatenate([[0], np.cumsum(nb)])

    IDX = np.zeros((N_CORES, NB * P), dtype=np.int16)
    S0 = np.full((N_CORES, P, NB), -1.0, dtype=np.float32)
    S1 = np.full((N_CORES, P, NB), -1.0, dtype=np.float32)
    C0 = np.zeros((N_CORES, P, NB), dtype=np.float32)
    C1 = np.zeros((N_CORES, P, NB), dtype=np.float32)

    for c in range(N_CORES):
        wc, kk, side, sc, cc, wbounds = per_core[c]
        for w in range(NW):
            lo, hi = wbounds[w], wbounds[w + 1]
            nslots = int(nb[w]) * P
            d_idx = np.zeros(nslots, dtype=np.int16)
            s0 = np.full(nslots, -1.0, dtype=np.float32)
            s1 = np.full(nslots, -1.0, dtype=np.float32)
            c0a = np.zeros(nslots, dtype=np.float32)
            c1a = np.zeros(nslots, dtype=np.float32)
            if hi > lo:
                k_w, s_w = kk[lo:hi], side[lo:hi]
                sl_w, scw = sc[lo:hi], cc[lo:hi]
                o2 = np.lexsort((s_w, k_w))
                k_s, s_s, sl_s, sc_s = k_w[o2], s_w[o2], sl_w[o2], scw[o2]
                uk, first, inv = np.unique(k_s, return_index=True,
                                           return_inverse=True)
                # rank within (k, side)
                grp_start_side = np.lexsort((np.arange(len(k_s)),))
                # cumcount within (k, side): since sorted by (k, side),
                # rank = position - first occurrence of (k, side)
                ks_key = k_s * 2 + s_s
                uks, first_ks, inv_ks = np.unique(ks_key, return_index=True,
                                                  return_inverse=True)
                rank = np.arange(len(k_s)) - first_ks[inv_ks]
                c0 = np.add.reduceat((s_s == 0).astype(np.int64), first)
                c1 = np.add.reduceat((s_s == 1).astype(np.int64), first)
                per_k = np.maximum(c0, c1)
                base = np.concatenate([[0], np.cumsum(per_k)[:-1]])
                di = base[inv] + rank
                nd = int(per_k.sum())
                assert nd <= nslots
                # descriptor index: expand uk over per_k
                d_idx[:nd] = np.repeat(uk, per_k).astype(np.int16)
                if nd < nslots:
                    d_idx[nd:] = d_idx[nd - 1] if nd > 0 else 0
                m0, m1 = s_s == 0, s_s == 1
                s0[di[m0]] = sl_s[m0]
                c0a[di[m0]] = sc_s[m0]
                s1[di[m1]] = sl_s[m1]
                c1a[di[m1]] = sc_s[m1]
            b0 = int(cum[w])
            IDX[c, b0 * P:(b0 + int(nb[w])) * P] = d_idx
            S0[c, :, b0:b0 + int(nb[w])] = s0.reshape(-1, P).T
            S1[c, :, b0:b0 + int(nb[w])] = s1.reshape(-1, P).T
            C0[c, :, b0:b0 + int(nb[w])] = c0a.reshape(-1, P).T
            C1[c, :, b0:b0 + int(nb[w])] = c1a.reshape(-1, P).T

    # ---- per-core tables and own-node data ----
    nf32 = np.asarray(node_features, dtype=np.float32)
    nf_pad = np.zeros((NTOT, F), dtype=ml_dtypes.bfloat16)
    nf_pad[:n_nodes] = nf32.astype(ml_dtypes.bfloat16)
    NFP = np.stack([nf_pad[tabperm[c]] for c in range(N_CORES)])

    # own rows pre-scaled by 0.5*deg, in (core, window, slot) order
    XOT = np.zeros((N_CORES, NPC, F), dtype=ml_dtypes.bfloat16)
    rows = np.zeros((NTOT, F), dtype=np.float32)
    degp = np.zeros(NTOT, dtype=np.float64)
    rows[node_row[:n_nodes]] = nf32 * (0.5 * deg[:n_nodes])[:, None]
    degp[node_row[:n_nodes]] = deg[:n_nodes]
    for c in range(N_CORES):
        XOT[c] = rows[c * NPC:(c + 1) * NPC].astype(ml_dtypes.bfloat16)

    IDXw = np.stack([_wrap16(IDX[c]) for c in range(N_CORES)])

    return dict(IDX=IDXw, S0=S0, S1=S1, C0=C0, C1=C1, NFP=NFP, XOT=XOT,
                nb=nb, cum=cum, NB=NB, node_row=node_row, n_nodes=n_nodes,
                deg=deg, degp=degp)


def _build_nc(hd, has_b):
    import concourse.bass as bass
    import concourse.bacc as bacc
    import concourse.mybir as mybir
    import concourse.tile as tile

    nb, cum, NB = hd["nb"], hd["cum"], hd["NB"]
    fp32 = mybir.dt.float32
    bf16 = mybir.dt.bfloat16

    nc = bacc.Bacc("TRN2", target_bir_lowering=False, debug=False,
                   num_swdge_queues=4)

    nfp_d = nc.dram_tensor("NFP", [NTOT // 2, 2 * F], bf16, kind="ExternalInput")
    xot_d = nc.dram_tensor("XOT", [NPC, F], bf16, kind="ExternalInput")
    idx_d = nc.dram_tensor("IDX", [P, NB * 8], mybir.dt.int16, kind="ExternalInput")
    s0_d = nc.dram_tensor("S0", [P, NB], fp32, kind="ExternalInput")
    s1_d = nc.dram_tensor("S1", [P, NB], fp32, kind="ExternalInput")
    c0_d = nc.dram_tensor("C0", [P, NB], fp32, kind="ExternalInput")
    c1_d = nc.dram_tensor("C1", [P, NB], fp32, kind="ExternalInput")
    iota_d = nc.dram_tensor("IOTA", [P, P], bf16, kind="ExternalInput")
    w_d = nc.dram_tensor("WM", [F, H], bf16, kind="ExternalInput")
    gp_d = nc.dram_tensor("GPCOL", [P, 1], fp32, kind="ExternalInput")
    bb_d = nc.dram_tensor("BBCOL", [P, 1], fp32, kind="ExternalInput")
    if has_b:
        brow_d = nc.dram_tensor("BROW", [1, H], bf16, kind="ExternalInput")
        sbrow_d = nc.dram_tensor("SBROW", [1, NPC], bf16, kind="ExternalInput")
    out_d = nc.dram_tensor("OUT_T", [P, NPC], fp32, kind="ExternalOutput")

    with tile.TileContext(nc) as tc:
        with (
            tc.tile_pool(name="meta", bufs=1) as meta,
            tc.tile_pool(name="g", bufs=8) as gpool,
            tc.tile_pool(name="o", bufs=8) as opool,
            tc.tile_pool(name="x", bufs=3) as xpool,
            tc.tile_pool(name="z", bufs=3) as zpool,
            tc.tile_pool(name="slab", bufs=1) as slab,
            tc.tile_pool(name="psz", bufs=2, space="PSUM") as psZ,
            tc.tile_pool(name="psnb", bufs=2, space="PSUM") as psNB,
        ):
            idx_sb = meta.tile([P, NB * 8], mybir.dt.int16)
            s0_sb = meta.tile([P, NB], fp32)
            s1_sb = meta.tile([P, NB], fp32)
            c0_sb = meta.tile([P, NB], fp32)
            c1_sb = meta.tile([P, NB], fp32)
            iota_sb = meta.tile([P, P], bf16)
            w_sb = meta.tile([F, H], bf16)
            gp_sb = meta.tile([P, 1], fp32)
            bb_sb = meta.tile([P, 1], fp32)

            c1cols = CHB * 8
            nc.sync.dma_start(idx_sb[:, :min(c1cols, NB * 8)],
                              idx_d[:, :min(c1cols, NB * 8)])
            if NB * 8 > c1cols:
                nc.sync.dma_start(idx_sb[:, c1cols:NB * 8], idx_d[:, c1cols:])
            nc.sync.dma_start(s0_sb[:], s0_d[:])
            nc.sync.dma_start(s1_sb[:], s1_d[:])
            nc.sync.dma_start(c0_sb[:], c0_d[:])
            nc.sync.dma_start(c1_sb[:], c1_d[:])
            nc.sync.dma_start(iota_sb[:], iota_d[:])
            nc.sync.dma_start(w_sb[:], w_d[:])
            nc.sync.dma_start(gp_sb[:], gp_d[:])
            nc.sync.dma_start(bb_sb[:], bb_d[:])
            if has_b:
                brow_sb = meta.tile([1, H], bf16)
                sbrow_sb = meta.tile([1, NPC], bf16)
                nc.sync.dma_start(brow_sb[:], brow_d[:])
                nc.sync.dma_start(sbrow_sb[:], sbrow_d[:])

            outT_sb = slab.tile([P, NPC], fp32)

            # ---- emit all gathers up front in consumption order ----
            nchunks = math.ceil(NB / CHB)
            gtiles = {}
            for ci in range(nchunks):
                b0, b1 = ci * CHB, min((ci + 1) * CHB, NB)
                nbc = b1 - b0
                gt = gpool.tile([P, nbc, 2 * F], bf16, tag="g")
                nidx = nbc * P
                nc.gpsimd.dma_gather(
                    gt[:], nfp_d[:], idx_sb[:, b0 * 8:b1 * 8],
                    nidx, nidx, 2 * F, single_packet=False, queue_num=0)
                gtiles[ci] = (b0, gt)

            def gslice(j, side):
                b0, gt = gtiles[j // CHB]
                return gt[:, j - b0, side * F:(side + 1) * F]

            def oslice(j, side):
                s_sb = s0_sb if side == 0 else s1_sb
                c_sb = c0_sb if side == 0 else c1_sb
                ot = opool.tile([P, P], bf16, tag="o")
                nc.vector.tensor_scalar(
                    out=ot[:], in0=iota_sb[:],
                    scalar1=s_sb[:, j:j + 1], scalar2=c_sb[:, j:j + 1],
                    op0=mybir.AluOpType.is_equal, op1=mybir.AluOpType.mult)
                return ot

            # ---- main window loop ----
            out_dma_step = max(1, NW // 8)
            for w in range(NW):
                nbw = int(nb[w])
                # transposed pre-scaled own rows (HWDGE xbar transpose)
                xt = xpool.tile([P, P], bf16, tag="xt")
                nc.sync.dma_start_transpose(xt[:], xot_d[w * P:(w + 1) * P, :])

                zt = None
                if nbw > 0:
                    psa = psZ.tile([P, P], fp32)
                    nmm = 2 * nbw
                    k = 0
                    for j in range(int(cum[w]), int(cum[w + 1])):
                        for side in (0, 1):
                            nc.tensor.matmul(psa[:], lhsT=gslice(j, side),
                                             rhs=oslice(j, side)[:],
                                             start=(k == 0), stop=(k == nmm - 1))
                            k += 1
                    zt = zpool.tile([P, P], bf16, tag="z")
                    nc.scalar.copy(zt[:], psa[:])

                psnb = psNB.tile([P, P], fp32)
                first = True
                if zt is not None:
                    nc.tensor.matmul(psnb[:], lhsT=w_sb[:], rhs=zt[:],
                                     start=True, stop=False)
                    first = False
                nc.tensor.matmul(psnb[:], lhsT=w_sb[:], rhs=xt[:],
                                 start=first, stop=not has_b)
                if has_b:
                    nc.tensor.matmul(psnb[:], lhsT=brow_sb[:],
                                     rhs=sbrow_sb[:, w * P:(w + 1) * P],
                                     start=False, stop=True)

                # BN affine (per-partition in feature-major layout)
                nc.scalar.activation(
                    outT_sb[:, w * P:(w + 1) * P], psnb[:],
                    mybir.ActivationFunctionType.Identity,
                    bias=bb_sb[:], scale=gp_sb[:],
                )

                if (w + 1) % out_dma_step == 0 or w == NW - 1:
                    lo = (w // out_dma_step) * out_dma_step
                    nc.sync.dma_start(out_d[:, lo * P:(w + 1) * P],
                                      outT_sb[:, lo * P:(w + 1) * P])

    # SWDGE queue ownership: each DMASW sem lane is owned by one queue, so
    # set queue_num = lane % num_queues after Tile assigned lanes.
    import concourse.mybir as mybir2
    from concourse.tile_scheduler import PROC_NAME_TO_IDX
    idx_to_proc = {v: k for k, v in PROC_NAME_TO_IDX.items()}
    for bb_ in nc.main_func.blocks:
        for ins in bb_.instructions:
            if isinstance(ins, mybir2.InstDMAGatherAnt):
                proc = idx_to_proc.get(ins.bass_scheduled_proc, "")
                if proc.startswith("DMASW"):
                    ins.queue_num = int(proc[5:]) % 4

    nc.compile()
    return nc


def _prepare(edge_pairs, node_features, W, b, gamma, beta, moving_mean, moving_var):
    hd = _build_host_data(edge_pairs, node_features)
    has_b = bool(np.any(np.asarray(b) != 0))

    key = (hd["n_nodes"], hd["NB"], tuple(hd["nb"].tolist()), has_b)
    if key not in _CACHE:
        _CACHE.clear()
        _CACHE[key] = _build_nc(hd, has_b)
    nc = _CACHE[key]

    gp = (np.asarray(gamma, np.float64)
          / np.sqrt(np.asarray(moving_var, np.float64) + BN_EPS))
    bb = np.asarray(beta, np.float64) - np.asarray(moving_mean, np.float64) * gp

    iota = np.tile(np.arange(P, dtype=np.float32).astype(ml_dtypes.bfloat16),
                   (P, 1))
    wmat = np.asarray(W, np.float32).astype(ml_dtypes.bfloat16)

    in_maps = []
    for c in range(N_CORES):
        m = {
            "NFP": np.ascontiguousarray(hd["NFP"][c].reshape(NTOT // 2, 2 * F)),
            "XOT": np.ascontiguousarray(hd["XOT"][c]),
            "IDX": np.ascontiguousarray(hd["IDX"][c]),
            "S0": np.ascontiguousarray(hd["S0"][c]),
            "S1": np.ascontiguousarray(hd["S1"][c]),
            "C0": np.ascontiguousarray(hd["C0"][c]),
            "C1": np.ascontiguousarray(hd["C1"][c]),
            "IOTA": iota,
            "WM": wmat,
            "GPCOL": gp.astype(np.float32).reshape(P, 1).copy(),
            "BBCOL": bb.astype(np.float32).reshape(P, 1).copy(),
        }
        if has_b:
            # b contribution: (0.5*isd_s*sum_e isd_d + 0.5*deg_s) * b
            deg = hd["deg"]
            src = np.asarray(edge_pairs[:, 0], dtype=np.int64)
            dstv = np.asarray(edge_pairs[:, 1], dtype=np.int64)
            with np.errstate(divide="ignore"):
                isd = 1.0 / np.sqrt(deg)
            ssum = np.bincount(src, weights=isd[dstv], minlength=hd["n_nodes"])
            sb_node = 0.5 * isd[:hd["n_nodes"]] * ssum + 0.5 * deg[:hd["n_nodes"]]
            sbrow = np.zeros(NTOT, dtype=np.float64)
            sbrow[hd["node_row"][:hd["n_nodes"]]] = sb_node
            m["BROW"] = np.asarray(b, np.float32).astype(
                ml_dtypes.bfloat16).reshape(1, H).copy()
            m["SBROW"] = sbrow[c * NPC:(c + 1) * NPC].astype(
                ml_dtypes.bfloat16).reshape(1, NPC).copy()
        in_maps.append(m)
    return nc, in_maps, hd


def _run(inputs, trace=False):
    from concourse.bass_utils import run_bass_kernel_spmd

    nc, in_maps, hd = _prepare(**inputs)
    res = run_bass_kernel_spmd(nc, in_maps, core_ids=list(range(N_CORES)),
                               trace=trace)
    full = np.empty((NTOT, H), dtype=np.float32)
    for c in range(N_CORES):
        full[c * NPC:(c + 1) * NPC] = res.results[c]["OUT_T"].T
    n = hd["n_nodes"]
    out = full[hd["node_row"][:n]]
    return np.ascontiguousarray(out), res


def kernel(**inputs):
    out, _ = _run(inputs, trace=False)
    return out


def run_traced(**inputs):
    return _run(inputs, trace=True)


# revision 5
# speedup vs baseline: 1.2605x; 1.2605x over previous
"""GCN encoder layer (degree-normalized message passing + BN inference) on 8 Trainium2 cores.

Math (see reference):
    t = X @ W + b                                  [N, H]
    deg = out-degree by src                        [N]
    isd = deg ** -0.5
    nb_sum[i]  = isd[i] * sum_{e: src=i} isd[dst_e] * t[dst_e]
    src_mean   = deg * t            (segment_mean(deg[src]*t[src]) simplifies exactly)
    agg = 0.5*nb_sum + 0.5*src_mean
    out = (agg - mean) * rsqrt(var+eps) * gamma + beta

Strategy (edge-parallel, sharded by src bucket -> no cross-core reduction):
  - Src nodes are assigned to 392 (core, window) buckets of 128 slots each by
    snake order on out-degree, equalizing edges per bucket (and per core).
  - The whole pipeline runs in bf16 (PSUM accumulation fp32); tolerance 2e-2.
  - The dst gather is descriptor-rate limited (~0.38 rows/ns over 4 SWDGE
    queues), so descriptors are the currency: each descriptor fetches a PAIR
    of adjacent bf16 table rows (elem 512B).  A per-core table permutation
    places nodes that are co-used in the same window in the same pair-slot, so
    one descriptor serves up to two edges (G0 = even row, G1 = odd row).
    Pair-slot indices (< 25088) fit int16 with no table split.
  - Scatter-add via one-hot matmuls on the PE:  Z_T[f, s] += G0.T @ O0 +
    G1.T @ O1 per descriptor batch, chained in one PSUM bank per window.
    O_side = (iota == srcl_side) * scl_side built in ONE fused DVE
    tensor_scalar (op0=is_equal, op1=mult) per batch side.
  - Source term: host pre-scales own rows by 0.5*deg; an HWDGE transpose-DMA
    delivers them feature-major, added via a second W matmul.
  - nb_T = W.T @ (zt) + W.T @ xT accumulated in PSUM; BN affine on ACT while
    copying to the output slab (feature-major; transposed on the host).
"""

import math
import numpy as np
import ml_dtypes

N_CORES = 8
P = 128
F = 128
H = 128
BN_EPS = 1e-3
NW = 49                 # windows per core
NPC = NW * P            # 6272 src slots per core
NTOT = N_CORES * NPC    # 50176 node slots (incl. padding)
CHB = 16                # gather chunk size in batches (16*128 descs)

_CACHE = {}


def _wrap16(arr):
    """dma_gather index layout: unwrapped[i] = w[i%16, i//16], replicated x8."""
    w = arr.reshape(-1, 16).T.copy()
    return np.ascontiguousarray(np.tile(w, (8, 1)))


def _build_host_data(edge_pairs, node_features):
    n_nodes = node_features.shape[0]
    src = np.asarray(edge_pairs[:, 0], dtype=np.int64)
    dst = np.asarray(edge_pairs[:, 1], dtype=np.int64)
    deg = np.bincount(src, minlength=n_nodes).astype(np.float64)

    # ---- bucket assignment: snake on degree over 392 buckets of 128 ----
    nb_buckets = N_CORES * NW
    order = np.argsort(-deg, kind="stable")          # node ids, deg desc
    bucket_of_rank = np.empty(NTOT, dtype=np.int64)
    fwd = np.arange(nb_buckets)
    for r in range(P):
        row = fwd if r % 2 == 0 else fwd[::-1]
        bucket_of_rank[r * nb_buckets:(r + 1) * nb_buckets] = row
    node_bucket = np.full(NTOT, -1, dtype=np.int64)
    node_slot = np.full(NTOT, -1, dtype=np.int64)
    padded_nodes = np.concatenate([order, np.arange(n_nodes, NTOT)])
    node_bucket[padded_nodes] = bucket_of_rank
    # slot within bucket = rank of appearance
    cnt = np.zeros(nb_buckets, dtype=np.int64)
    slot_of_rank = np.empty(NTOT, dtype=np.int64)
    for i, b in enumerate(bucket_of_rank):
        slot_of_rank[i] = cnt[b]
        cnt[b] += 1
    node_slot[padded_nodes] = slot_of_rank
    node_core = node_bucket % N_CORES
    node_win = node_bucket // N_CORES

    # inverse map for output unshard: node -> global row in [NTOT]
    node_row = node_core * NPC + node_win * P + node_slot

    # ---- per-edge metadata ----
    core_e = node_core[src]
    win_e = node_win[src]
    srcl_e = node_slot[src]
    scl_e = np.zeros(len(src), dtype=np.float32)  # unused (scales folded)

    IDXs, S0s, C0s, S1s, C1s = [], [], [], [], []
    tabperm = np.empty((N_CORES, NTOT), dtype=np.int64)
    ndesc_cw = np.zeros((N_CORES, NW), dtype=np.int64)
    per_core = []
    for c in range(N_CORES):
        m = core_e == c
        wc, dc, sc, cc = win_e[m], dst[m], srcl_e[m], scl_e[m]
        # ---- greedy pair matching over windows ----
        partner = np.full(n_nodes, -1, dtype=np.int64)
        o = np.lexsort((dc, wc))
        wc, dc, sc, cc = wc[o], dc[o], sc[o], cc[o]
        wbounds = np.searchsorted(wc, np.arange(NW + 1))
        for w in range(NW):
            dw = np.unique(dc[wbounds[w]:wbounds[w + 1]])
            cand = dw[partner[dw] < 0]
            k = len(cand) // 2 * 2
            a, b = cand[0:k:2], cand[1:k:2]
            partner[a] = b
            partner[b] = a
        # ---- per-core table permutation: pairs adjacent ----
        nodes = np.arange(n_nodes)
        isp = partner >= 0
        a_nodes = nodes[isp & (nodes < partner)]
        singles = nodes[~isp]
        npad = NTOT - n_nodes
        ordert = np.empty(NTOT, dtype=np.int64)
        na = len(a_nodes)
        ordert[0:2 * na:2] = a_nodes
        ordert[1:2 * na:2] = partner[a_nodes]
        ordert[2 * na:2 * na + len(singles)] = singles
        ordert[2 * na + len(singles):] = np.arange(n_nodes, NTOT)
        tabperm[c] = ordert
        pos = np.empty(NTOT, dtype=np.int64)
        pos[ordert] = np.arange(NTOT)
        # ---- descriptor assembly per window ----
        kk = pos[dc] // 2
        side = pos[dc] % 2
        per_core.append((wc, kk, side, sc, cc, wbounds))

    # window batch counts (shared across cores -> max)
    for c in range(N_CORES):
        wc, kk, side, sc, cc, wbounds = per_core[c]
        for w in range(NW):
            lo, hi = wbounds[w], wbounds[w + 1]
            if hi == lo:
                continue
            k_w, s_w = kk[lo:hi], side[lo:hi]
            o2 = np.lexsort((s_w, k_w))
            k_s, s_s = k_w[o2], s_w[o2]
            uk, first = np.unique(k_s, return_index=True)
            c0 = np.add.reduceat((s_s == 0).astype(np.int64), first)
            c1 = np.add.reduceat((s_s == 1).astype(np.int64), first)
            ndesc_cw[c, w] = np.maximum(c0, c1).sum()
    nb = np.ceil(ndesc_cw.max(axis=0) / P).astype(np.int64)   # [NW]
    NB = int(nb.sum())
    cum = np.concatenate([[0], np.cumsum(nb)])

    IDX = np.zeros((N_CORES, NB * P), dtype=np.int16)
    S0 = np.full((N_CORES, P, NB), -1.0, dtype=ml_dtypes.bfloat16)
    S1 = np.full((N_CORES, P, NB), -1.0, dtype=ml_dtypes.bfloat16)

    for c in range(N_CORES):
        wc, kk, side, sc, cc, wbounds = per_core[c]
        for w in range(NW):
            lo, hi = wbounds[w], wbounds[w + 1]
            nslots = int(nb[w]) * P
            d_idx = np.zeros(nslots, dtype=np.int16)
            s0 = np.full(nslots, -1.0, dtype=np.float32)
            s1 = np.full(nslots, -1.0, dtype=np.float32)
            if hi > lo:
                k_w, s_w = kk[lo:hi], side[lo:hi]
                sl_w, scw = sc[lo:hi], cc[lo:hi]
                o2 = np.lexsort((s_w, k_w))
                k_s, s_s, sl_s, sc_s = k_w[o2], s_w[o2], sl_w[o2], scw[o2]
                uk, first, inv = np.unique(k_s, return_index=True,
                                           return_inverse=True)
                # rank within (k, side)
                grp_start_side = np.lexsort((np.arange(len(k_s)),))
                # cumcount within (k, side): since sorted by (k, side),
                # rank = position - first occurrence of (k, side)
                ks_key = k_s * 2 + s_s
                uks, first_ks, inv_ks = np.unique(ks_key, return_index=True,
                                                  return_inverse=True)
                rank = np.arange(len(k_s)) - first_ks[inv_ks]
                c0 = np.add.reduceat((s_s == 0).astype(np.int64), first)
                c1 = np.add.reduceat((s_s == 1).astype(np.int64), first)
                per_k = np.maximum(c0, c1)
                base = np.concatenate([[0], np.cumsum(per_k)[:-1]])
                di = base[inv] + rank
                nd = int(per_k.sum())
                assert nd <= nslots
                # descriptor index: expand uk over per_k
                d_idx[:nd] = np.repeat(uk, per_k).astype(np.int16)
                if nd < nslots:
                    d_idx[nd:] = d_idx[nd - 1] if nd > 0 else 0
                m0, m1 = s_s == 0, s_s == 1
                s0[di[m0]] = sl_s[m0]
                s1[di[m1]] = sl_s[m1]
            b0 = int(cum[w])
            IDX[c, b0 * P:(b0 + int(nb[w])) * P] = d_idx
            S0[c, :, b0:b0 + int(nb[w])] = s0.reshape(-1, P).T.astype(ml_dtypes.bfloat16)
            S1[c, :, b0:b0 + int(nb[w])] = s1.reshape(-1, P).T.astype(ml_dtypes.bfloat16)

    # ---- per-core tables (rows pre-scaled by 0.5*isd_dst) ----
    nf32 = np.asarray(node_features, dtype=np.float32)
    with np.errstate(divide="ignore"):
        isd = 1.0 / np.sqrt(deg)
    nf_pad = np.zeros((NTOT, F), dtype=ml_dtypes.bfloat16)
    nf_pad[:n_nodes] = (nf32 * (0.5 * isd[:n_nodes])[:, None]).astype(
        ml_dtypes.bfloat16)
    NFP = np.stack([nf_pad[tabperm[c]] for c in range(N_CORES)])

    # own rows pre-scaled by 0.5*deg^1.5 (so the final per-column isd_s
    # scale turns this into 0.5*deg), in (core, window, slot) order
    XOT = np.zeros((N_CORES, NPC, F), dtype=ml_dtypes.bfloat16)
    rows = np.zeros((NTOT, F), dtype=np.float32)
    rows[node_row[:n_nodes]] = nf32 * (0.5 * deg[:n_nodes] ** 1.5)[:, None]
    for c in range(N_CORES):
        XOT[c] = rows[c * NPC:(c + 1) * NPC].astype(ml_dtypes.bfloat16)

    # per-column isd_s (0 for deg-0/padding slots), replicated to 128 rows
    isdr = np.zeros(NTOT, dtype=np.float32)
    good = deg[:n_nodes] > 0
    isdr[node_row[:n_nodes][good]] = isd[:n_nodes][good].astype(np.float32)
    ISDM = np.zeros((N_CORES, P, NPC), dtype=np.float32)
    for c in range(N_CORES):
        ISDM[c] = np.tile(isdr[c * NPC:(c + 1) * NPC], (P, 1))

    IDXw = np.stack([_wrap16(IDX[c]) for c in range(N_CORES)])

    return dict(IDX=IDXw, S0=S0, S1=S1, NFP=NFP, XOT=XOT, ISDM=ISDM,
                nb=nb, cum=cum, NB=NB, node_row=node_row, n_nodes=n_nodes,
                deg=deg)


def _build_nc(hd, has_b):
    import concourse.bass as bass
    import concourse.bacc as bacc
    import concourse.mybir as mybir
    import concourse.tile as tile

    nb, cum, NB = hd["nb"], hd["cum"], hd["NB"]
    fp32 = mybir.dt.float32
    bf16 = mybir.dt.bfloat16

    nc = bacc.Bacc("TRN2", target_bir_lowering=False, debug=False,
                   num_swdge_queues=4)

    nfp_d = nc.dram_tensor("NFP", [NTOT // 2, 2 * F], bf16, kind="ExternalInput")
    xot_d = nc.dram_tensor("XOT", [NPC, F], bf16, kind="ExternalInput")
    idx_d = nc.dram_tensor("IDX", [P, NB * 8], mybir.dt.int16, kind="ExternalInput")
    s0_d = nc.dram_tensor("S0", [P, NB], bf16, kind="ExternalInput")
    s1_d = nc.dram_tensor("S1", [P, NB], bf16, kind="ExternalInput")
    isdm_d = nc.dram_tensor("ISDM", [P, NPC], fp32, kind="ExternalInput")
    iota_d = nc.dram_tensor("IOTA8", [P, 8 * P], bf16, kind="ExternalInput")
    w_d = nc.dram_tensor("WM", [F, H], bf16, kind="ExternalInput")
    gp_d = nc.dram_tensor("GPCOL", [P, 1], fp32, kind="ExternalInput")
    bb_d = nc.dram_tensor("BBCOL", [P, 1], fp32, kind="ExternalInput")
    if has_b:
        brow_d = nc.dram_tensor("BROW", [1, H], bf16, kind="ExternalInput")
        sbrow_d = nc.dram_tensor("SBROW", [1, NPC], bf16, kind="ExternalInput")
    out_d = nc.dram_tensor("OUT_T", [P, NPC], fp32, kind="ExternalOutput")

    with tile.TileContext(nc) as tc:
        with (
            tc.tile_pool(name="meta", bufs=1) as meta,
            tc.tile_pool(name="g", bufs=8) as gpool,
            tc.tile_pool(name="o", bufs=8) as opool,
            tc.tile_pool(name="x", bufs=3) as xpool,
            tc.tile_pool(name="z", bufs=3) as zpool,
            tc.tile_pool(name="slab", bufs=1) as slab,
            tc.tile_pool(name="psz", bufs=2, space="PSUM") as psZ,
            tc.tile_pool(name="psnb", bufs=2, space="PSUM") as psNB,
        ):
            idx_sb = meta.tile([P, NB * 8], mybir.dt.int16)
            s0_sb = meta.tile([P, NB], bf16)
            s1_sb = meta.tile([P, NB], bf16)
            isdm_sb = meta.tile([P, NPC], fp32)
            iota_sb = meta.tile([P, 8 * P], bf16)
            w_sb = meta.tile([F, H], bf16)
            gp_sb = meta.tile([P, 1], fp32)
            bb_sb = meta.tile([P, 1], fp32)

            c1cols = CHB * 8
            nc.sync.dma_start(idx_sb[:, :min(c1cols, NB * 8)],
                              idx_d[:, :min(c1cols, NB * 8)])
            if NB * 8 > c1cols:
                nc.sync.dma_start(idx_sb[:, c1cols:NB * 8], idx_d[:, c1cols:])
            nc.sync.dma_start(s0_sb[:], s0_d[:])
            nc.sync.dma_start(s1_sb[:], s1_d[:])
            nc.sync.dma_start(isdm_sb[:], isdm_d[:])
            nc.sync.dma_start(iota_sb[:], iota_d[:])
            nc.sync.dma_start(w_sb[:], w_d[:])
            nc.sync.dma_start(gp_sb[:], gp_d[:])
            nc.sync.dma_start(bb_sb[:], bb_d[:])
            if has_b:
                brow_sb = meta.tile([1, H], bf16)
                sbrow_sb = meta.tile([1, NPC], bf16)
                nc.sync.dma_start(brow_sb[:], brow_d[:])
                nc.sync.dma_start(sbrow_sb[:], sbrow_d[:])

            outT_sb = slab.tile([P, NPC], fp32)

            # ---- emit all gathers up front in consumption order ----
            nchunks = math.ceil(NB / CHB)
            gtiles = {}
            for ci in range(nchunks):
                b0, b1 = ci * CHB, min((ci + 1) * CHB, NB)
                nbc = b1 - b0
                gt = gpool.tile([P, nbc, 2 * F], bf16, tag="g")
                nidx = nbc * P
                nc.gpsimd.dma_gather(
                    gt[:], nfp_d[:], idx_sb[:, b0 * 8:b1 * 8],
                    nidx, nidx, 2 * F, single_packet=False, queue_num=0)
                gtiles[ci] = (b0, gt)

            def gslice(j, side):
                b0, gt = gtiles[j // CHB]
                return gt[:, j - b0, side * F:(side + 1) * F]

            GRP = 8
            ogroups = {}

            def oslice(j, side):
                g = j // GRP
                key = (g, side)
                if key not in ogroups:
                    s_sb = s0_sb if side == 0 else s1_sb
                    g0 = g * GRP
                    m = min(GRP, NB - g0)
                    o8 = opool.tile([P, m * P], bf16, tag="o")
                    nc.vector.tensor_tensor(
                        out=o8[:], in0=iota_sb[:, :m * P],
                        in1=s_sb[:, g0:g0 + m].to_broadcast([P, m, P]),
                        op=mybir.AluOpType.is_equal)
                    ogroups[key] = o8
                k = j % GRP
                return ogroups[key][:, k * P:(k + 1) * P]

            # ---- main window loop ----
            out_dma_step = max(1, NW // 8)
            for w in range(NW):
                nbw = int(nb[w])
                # transposed pre-scaled own rows (HWDGE xbar transpose)
                xt = xpool.tile([P, P], bf16, tag="xt")
                nc.sync.dma_start_transpose(xt[:], xot_d[w * P:(w + 1) * P, :])

                zt = None
                if nbw > 0:
                    psa = psZ.tile([P, P], fp32)
                    nmm = 2 * nbw
                    k = 0
                    for j in range(int(cum[w]), int(cum[w + 1])):
                        for side in (0, 1):
                            nc.tensor.matmul(psa[:], lhsT=gslice(j, side),
                                             rhs=oslice(j, side)[:],
                                             start=(k == 0), stop=(k == nmm - 1))
                            k += 1
                    zt = zpool.tile([P, P], bf16, tag="z")
                    nc.scalar.copy(zt[:], psa[:])

                psnb = psNB.tile([P, P], fp32)
                first = True
                if zt is not None:
                    nc.tensor.matmul(psnb[:], lhsT=w_sb[:], rhs=zt[:],
                                     start=True, stop=False)
                    first = False
                nc.tensor.matmul(psnb[:], lhsT=w_sb[:], rhs=xt[:],
                                 start=first, stop=not has_b)
                if has_b:
                    nc.tensor.matmul(psnb[:], lhsT=brow_sb[:],
                                     rhs=sbrow_sb[:, w * P:(w + 1) * P],
                                     start=False, stop=True)

                # per-column isd_s scale (DVE), then BN affine (ACT)
                tmp = zpool.tile([P, P], fp32, tag="tmp")
                nc.vector.tensor_tensor(
                    out=tmp[:], in0=psnb[:],
                    in1=isdm_sb[:, w * P:(w + 1) * P],
                    op=mybir.AluOpType.mult)
                nc.scalar.activation(
                    outT_sb[:, w * P:(w + 1) * P], tmp[:],
                    mybir.ActivationFunctionType.Identity,
                    bias=bb_sb[:], scale=gp_sb[:],
                )

                if (w + 1) % out_dma_step == 0 or w == NW - 1:
                    lo = (w // out_dma_step) * out_dma_step
                    nc.sync.dma_start(out_d[:, lo * P:(w + 1) * P],
                                      outT_sb[:, lo * P:(w + 1) * P])

    # SWDGE queue ownership: each DMASW sem lane is owned by one queue, so
    # set queue_num = lane % num_queues after Tile assigned lanes.
    import concourse.mybir as mybir2
    from concourse.tile_scheduler import PROC_NAME_TO_IDX
    idx_to_proc = {v: k for k, v in PROC_NAME_TO_IDX.items()}
    for bb_ in nc.main_func.blocks:
        for ins in bb_.instructions:
            if isinstance(ins, mybir2.InstDMAGatherAnt):
                proc = idx_to_proc.get(ins.bass_scheduled_proc, "")
                if proc.startswith("DMASW"):
                    ins.queue_num = int(proc[5:]) % 4

    nc.compile()
    return nc


def _prepare(edge_pairs, node_features, W, b, gamma, beta, moving_mean, moving_var):
    hd = _build_host_data(edge_pairs, node_features)
    has_b = bool(np.any(np.asarray(b) != 0))

    key = (hd["n_nodes"], hd["NB"], tuple(hd["nb"].tolist()), has_b)
    if key not in _CACHE:
        _CACHE.clear()
        _CACHE[key] = _build_nc(hd, has_b)
    nc = _CACHE[key]

    gp = (np.asarray(gamma, np.float64)
          / np.sqrt(np.asarray(moving_var, np.float64) + BN_EPS))
    bb = np.asarray(beta, np.float64) - np.asarray(moving_mean, np.float64) * gp

    iota = np.tile(np.arange(P, dtype=np.float32).astype(ml_dtypes.bfloat16),
                   (P, 8))
    wmat = np.asarray(W, np.float32).astype(ml_dtypes.bfloat16)

    in_maps = []
    for c in range(N_CORES):
        m = {
            "NFP": np.ascontiguousarray(hd["NFP"][c].reshape(NTOT // 2, 2 * F)),
            "XOT": np.ascontiguousarray(hd["XOT"][c]),
            "IDX": np.ascontiguousarray(hd["IDX"][c]),
            "S0": np.ascontiguousarray(hd["S0"][c]),
            "S1": np.ascontiguousarray(hd["S1"][c]),
            "ISDM": np.ascontiguousarray(hd["ISDM"][c]),
            "IOTA8": iota,
            "WM": wmat,
            "GPCOL": gp.astype(np.float32).reshape(P, 1).copy(),
            "BBCOL": bb.astype(np.float32).reshape(P, 1).copy(),
        }
        if has_b:
            # b contribution: (0.5*isd_s*sum_e isd_d + 0.5*deg_s) * b
            deg = hd["deg"]
            src = np.asarray(edge_pairs[:, 0], dtype=np.int64)
            dstv = np.asarray(edge_pairs[:, 1], dtype=np.int64)
            with np.errstate(divide="ignore"):
                isd = 1.0 / np.sqrt(deg)
            ssum = np.bincount(src, weights=isd[dstv], minlength=hd["n_nodes"])
            sb_node = 0.5 * ssum + 0.5 * deg[:hd["n_nodes"]] ** 1.5
            sbrow = np.zeros(NTOT, dtype=np.float64)
            sbrow[hd["node_row"][:hd["n_nodes"]]] = sb_node
            m["BROW"] = np.asarray(b, np.float32).astype(
                ml_dtypes.bfloat16).reshape(1, H).copy()
            m["SBROW"] = sbrow[c * NPC:(c + 1) * NPC].astype(
                ml_dtypes.bfloat16).reshape(1, NPC).copy()
        in_maps.append(m)
    return nc, in_maps, hd


def _run(inputs, trace=False):
    from concourse.bass_utils import run_bass_kernel_spmd

    nc, in_maps, hd = _prepare(**inputs)
    res = run_bass_kernel_spmd(nc, in_maps, core_ids=list(range(N_CORES)),
                               trace=trace)
    full = np.empty((NTOT, H), dtype=np.float32)
    for c in range(N_CORES):
        full[c * NPC:(c + 1) * NPC] = res.results[c]["OUT_T"].T
    n = hd["n_nodes"]
    out = full[hd["node_row"][:n]]
    return np.ascontiguousarray(out), res


def kernel(**inputs):
    out, _ = _run(inputs, trace=False)
    return out


def run_traced(**inputs):
    return _run(inputs, trace=True)
